# revision 48
# baseline (speedup 1.0000x reference)
"""Bass/Trainium2 kernel for nn_MAC_30554397344312 (gnn_message_passing).

Reference computation (B=256 rollout groups, n=64 agents, D=256):
    comm = h @ W_act.T + b_act                      # (B*n, D)
    agg[b,j] = sum_i mask[i,j] * comm[b,i] / (n-1)  # mask = ones - eye
    x   = agg @ W_sum.T + b_sum
    out = relu(x @ W_head.T + b_head)

Everything before the relu is linear, so fold on host:
    Wc = W_head @ W_sum @ W_act          (256x256)
    bc = b_head + b_sum @ W_head.T + b_act @ (W_head @ W_sum).T
    out[b,j] = relu( (A @ H_b)[j] @ Wc.T + bc ),  A = (ones-eye)/(n-1)

On device (per core, 2048 rows = 16 token tiles of 128):
    loads  (SWDGE): h chunks DMA'd with inline f32 -> fp16 cast, so no
                    on-chip cast stage at all; loads own the gpsimd ring,
                    stores own the two HWDGE rings (R/W streams overlap).
    stage 1 (PE): Y.T tiles [d, tok] via matmul(lhsT=H_tile[128tok,128d],
                  rhs=blockdiag(A,A)) - aggregation and transpose fused.
    stage 2 (DVE): evict Y.T PSUM bank to SBUF (one [128,512] copy/chunk).
    stage 3 (PE): out[tok, d_out] = Y.T.T @ Wc.T accumulated over 2 k-chunks.
    stage 4 (ACT/DVE alternating): relu + scale + PSUM->SBUF evict.
    stage 5: per-chunk DMA store, alternating HWDGE rings.

A short burst of dependency-free warm-up matmuls runs right after the
preamble barrier so the PE HAM clock gate releases (1.2 -> 2.4 GHz)
before the real matmul stream arrives.

Sharding: data-parallel over the B axis, 8 cores x 2048 rows.
"""

from contextlib import ExitStack

import numpy as np

import concourse.bacc as bacc
import concourse.bass as bass
import concourse.tile as tile
from concourse import mybir
from concourse.bass_utils import run_bass_kernel_spmd

N_AGENTS = 64
B = 256
D = 256
N_CORES = 8
ROWS = B * N_AGENTS            # 16384
ROWS_PER_CORE = ROWS // N_CORES  # 2048
P = 128
N_TILES = ROWS_PER_CORE // P   # 16 token tiles per core
# chunk plan: (tile_start, n_tiles, transport); the head and tail chunks
# ride the otherwise-idle HWDGE rings as f32 (DVE casts them) so they land
# early; the middle 12 tiles stream via SWDGE with inline f32->fp16 cast
# (bigger chunks amortize its ~1.1us per-DMA floor).
# input chunks (tile_start, n_tiles, transport): the SWDGE ring streams 10
# middle tiles with inline f32->fp16 cast while the otherwise-idle HWDGE
# rings carry 6 tiles as f32 early (DVE casts them) - input fully lands by
# ~13.5us instead of ~16us.
CHUNKS = [(0, 2, "sw"), (2, 2, "sync"), (4, 4, "sw"), (8, 4, "sw"),
          (12, 2, "scalar"), (14, 2, "sync")]
MAXT = 4                       # max tiles per chunk (PSUM tile sizing)
# compute/store granularity: 2-tile units, decoupled from input chunking
UNITS = [(2 * u, 2) for u in range(8)]
# unit -> input chunk covering it
UNIT_CHUNK = [0, 1, 2, 2, 3, 3, 4, 5]
RELU_DVE = (3, 5)              # units whose relu runs on DVE (rest on ACT)
ST_SCALAR = (1, 6)             # units whose store issues from scalar
W_SCALE = 16.0  # fp16 weight prescale (power of 2; inverted exactly in relu)

_cache = {}


def _build(has_bias: bool, f16: bool = True):
    f32 = mybir.dt.float32
    mdt = mybir.dt.float16 if f16 else mybir.dt.float32
    inv_scale = 1.0 / W_SCALE if f16 else 1.0
    nc = bacc.Bacc("TRN2", target_bir_lowering=False, debug=False,
                   num_devices=N_CORES)

    h = nc.dram_tensor("h", [ROWS_PER_CORE, D], f32, kind="ExternalInput")
    wcT = nc.dram_tensor("wcT", [D, D], mdt, kind="ExternalInput")
    ablk = nc.dram_tensor("ablk", [P, P], mdt, kind="ExternalInput")
    if has_bias:
        bc = nc.dram_tensor("bc", [1, D], f32, kind="ExternalInput")
    out = nc.dram_tensor("out", [ROWS_PER_CORE, D], f32, kind="ExternalOutput")

    h_ap = h[:, :].rearrange("(n p) d -> p n d", p=P)      # [128, 16, 256]
    out_ap = out[:, :].rearrange("(n p) d -> p n d", p=P)  # [128, 16, 256]
    w_ap = wcT[:, :].rearrange("(k p) d -> p k d", p=P)    # [128, 2, 256]

    NCH = len(CHUNKS)

    with tile.TileContext(nc) as tc:
        with ExitStack() as ctx:
            const = ctx.enter_context(tc.tile_pool(name="const", bufs=1))
            aggps = ctx.enter_context(
                tc.tile_pool(name="aggps", bufs=2, space="PSUM"))
            outps = ctx.enter_context(
                tc.tile_pool(name="outps", bufs=3, space="PSUM"))

            a_t = const.tile([P, P], mdt, tag="a", name="a_t")
            w_t = const.tile([P, 2, D], mdt, tag="w", name="w_t")
            if has_bias:
                bc_t = const.tile([P, D], f32, tag="bc", name="bc_t")

            # ---- PE warm-up: dependency-free matmuls on scratch data so the
            # HAM clock gate releases (1.2 -> 2.4 GHz) before real work; the
            # burst bridges the input-DMA latency window (~3.4us busy needed).
            ws_t = const.tile([P, 4 * P], mdt, tag="ws", name="ws_t")
            nc.vector.memset(ws_t[:], 0.0)
            wp_t = outps.tile([P, 2, D], f32, tag="outps", name="wp_t")
            for _ in range(6):
                nc.tensor.matmul(wp_t[:], ws_t[:, :P],
                                 ws_t[:], start=True, stop=True)

            # ---- weights on the (otherwise store-only) HWDGE rings
            nc.sync.dma_start(out=a_t[:], in_=ablk[:, :])
            nc.scalar.dma_start(out=w_t[:], in_=w_ap)
            if has_bias:
                bc_bcast = bass.AP(tensor=bc, offset=0, ap=[[0, P], [1, D]])
                nc.gpsimd.dma_start(out=bc_t[:], in_=bc_bcast)

            # ---- input loads (mixed transport)
            hc = []
            traw = {}
            for c, (t0, nt, tr) in enumerate(CHUNKS):
                if tr == "sw" and f16:
                    t = const.tile([P, nt, D], mdt, tag=f"hc{c}",
                                   name=f"hc_{c}")
                    nc.gpsimd.dma_start(out=t[:], in_=h_ap[:, t0:t0 + nt, :])
                else:
                    r = const.tile([P, nt, D], f32, tag=f"hr{c}",
                                   name=f"hr_{c}")
                    eng = nc.scalar if tr == "scalar" else nc.sync
                    eng.dma_start(out=r[:], in_=h_ap[:, t0:t0 + nt, :])
                    if f16:
                        traw[c] = r
                        t = const.tile([P, nt, D], mdt, tag=f"hc{c}",
                                       name=f"hc_{c}")
                    else:
                        t = r
                hc.append(t)

            def cast(c):
                if c in traw:
                    nc.vector.tensor_copy(out=hc[c][:], in_=traw[c][:])

            # Y.T in SBUF: [128 d, 2 k-chunks, 2048 tok] single tile
            yt = const.tile([P, 2, ROWS_PER_CORE], mdt, tag="yt", name="yt")
            och = [const.tile([P, nt, D], f32, tag=f"oc{u}", name=f"oc_{u}")
                   for u, (t0, nt) in enumerate(UNITS)]

            def agg(c):
                t0, nt, _ = CHUNKS[c]
                # one PSUM region per chunk, k-major columns [k, s, 128]
                ps = aggps.tile([P, 2, MAXT * P], f32, tag="aggps",
                                name="agg_ps")
                ps = ps[:, :, :nt * P]
                for s in range(nt):
                    for k in range(2):
                        lhsT = hc[c][:, s, k * P:(k + 1) * P]
                        nc.tensor.matmul(
                            ps[:, k, s * P:(s + 1) * P], lhsT, a_t[:],
                            start=True, stop=True)
                # single DVE evict for the whole chunk (both k halves)
                nc.vector.tensor_copy(
                    out=yt[:, :, t0 * P:(t0 + nt) * P], in_=ps[:])

            def main(u):
                t0, nt = UNITS[u]
                po = outps.tile([P, 2, D], f32, tag="outps", name="po")
                po = po[:, :nt, :]
                for s in range(nt):
                    m = t0 + s
                    for k in range(2):
                        nc.tensor.matmul(
                            po[:, s, :], yt[:, k, m * P:(m + 1) * P],
                            w_t[:, k, :], start=(k == 0), stop=(k == 1))
                dst = och[u][:]
                if has_bias:
                    for s in range(nt):
                        nc.vector.tensor_scalar(
                            out=och[u][:, s, :], in0=po[:, s, :],
                            scalar1=inv_scale, scalar2=None,
                            op0=mybir.AluOpType.mult)
                        nc.vector.tensor_tensor(
                            out=och[u][:, s, :], in0=och[u][:, s, :],
                            in1=bc_t[:], op=mybir.AluOpType.add)
                        nc.scalar.activation(
                            out=och[u][:, s, :], in_=och[u][:, s, :],
                            func=mybir.ActivationFunctionType.Relu)
                elif u in RELU_DVE:
                    nc.vector.tensor_scalar(
                        out=dst, in0=po[:], scalar1=inv_scale,
                        scalar2=0.0, op0=mybir.AluOpType.mult,
                        op1=mybir.AluOpType.max)
                else:
                    nc.scalar.activation(
                        out=dst, in_=po[:],
                        func=mybir.ActivationFunctionType.Relu,
                        scale=inv_scale)
                (nc.scalar if u in ST_SCALAR else nc.sync).dma_start(
                    out=out_ap[:, t0:t0 + nt, :], in_=och[u][:])

            # process in expected arrival order: HWDGE chunks land first,
            # SWDGE middle chunks stream in behind them
            cast(1)
            cast(4)
            cast(5)
            agg(1)            # tiles 2-3   -> unit 1   (HWDGE, ~9.8)
            agg(4)            # tiles 12-13 -> unit 6   (HWDGE)
            agg(5)            # tiles 14-15 -> unit 7   (HWDGE)
            agg(0)            # tiles 0-1   -> unit 0   (SWDGE, ~10.3)
            main(1)
            main(6)
            main(7)
            agg(2)            # tiles 4-7   -> units 2,3 (SWDGE)
            main(0)
            main(2)
            main(3)
            agg(3)            # tiles 8-11  -> units 4,5 (SWDGE)
            main(4)
            main(5)
    nc.finalize()
    return nc


def _fold(W_act, b_act, W_sum, b_sum, W_head, b_head, f16=True):
    Wa = W_act.astype(np.float64)
    Ws = W_sum.astype(np.float64)
    Wh = W_head.astype(np.float64)
    Wc = Wh @ Ws @ Wa
    bc = (b_head.astype(np.float64)
          + b_sum.astype(np.float64) @ Wh.T
          + b_act.astype(np.float64) @ (Wh @ Ws).T)
    A = np.ones((N_AGENTS, N_AGENTS)) - np.eye(N_AGENTS)
    if f16:
        # mask stays exact 0/1 in fp16; 1/63 and the fp16-subnormal
        # prescale fold into the weights, inverted via the relu scale.
        WcT = (Wc.T / (N_AGENTS - 1) * W_SCALE).astype(np.float16)
        wdt = np.float16
    else:
        A = A / (N_AGENTS - 1)
        WcT = Wc.T.astype(np.float32)
        wdt = np.float32
    Ablk = np.zeros((P, P))
    Ablk[:N_AGENTS, :N_AGENTS] = A
    Ablk[N_AGENTS:, N_AGENTS:] = A
    return (np.ascontiguousarray(WcT), bc.astype(np.float32),
            Ablk.astype(wdt))


def kernel(hidden_state, W_act, b_act, W_sum, b_sum, W_head, b_head,
           _trace=False, _tmpdir=None):
    import os
    f16 = os.environ.get("KERNEL_F32", "0") != "1"
    h = np.ascontiguousarray(np.asarray(hidden_state, dtype=np.float32))
    WcT, bc, Ablk = _fold(np.asarray(W_act), np.asarray(b_act),
                          np.asarray(W_sum), np.asarray(b_sum),
                          np.asarray(W_head), np.asarray(b_head), f16=f16)
    has_bias = bool(np.any(bc))
    if (has_bias, f16) not in _cache:
        _cache[(has_bias, f16)] = _build(has_bias, f16=f16)
    nc = _cache[(has_bias, f16)]

    in_maps = []
    for c in range(N_CORES):
        m = {"h": h[c * ROWS_PER_CORE:(c + 1) * ROWS_PER_CORE],
             "wcT": WcT, "ablk": Ablk}
        if has_bias:
            m["bc"] = bc.reshape(1, D)
        in_maps.append(m)

    res = run_bass_kernel_spmd(
        nc, in_maps, core_ids=list(range(N_CORES)),
        trace=_trace, tmpdir=_tmpdir)
    out = np.concatenate([res.results[c]["out"] for c in range(N_CORES)],
                         axis=0)
    if _trace:
        return out, res
    return out


# revision 53
# speedup vs baseline: 1.0474x; 1.0474x over previous
"""Bass/Trainium2 kernel for nn_MAC_30554397344312 (gnn_message_passing).

Reference computation (B=256 rollout groups, n=64 agents, D=256):
    comm = h @ W_act.T + b_act                      # (B*n, D)
    agg[b,j] = sum_i mask[i,j] * comm[b,i] / (n-1)  # mask = ones - eye
    x   = agg @ W_sum.T + b_sum
    out = relu(x @ W_head.T + b_head)

Everything before the relu is linear, so fold on host:
    Wc = W_head @ W_sum @ W_act          (256x256)
    bc = b_head + b_sum @ W_head.T + b_act @ (W_head @ W_sum).T
    out[b,j] = relu( (A @ H_b)[j] @ Wc.T + bc ),  A = (ones-eye)/(n-1)

On device (per core, 2048 rows = 16 token tiles of 128):
    loads  (SWDGE): h chunks DMA'd with inline f32 -> fp16 cast, so no
                    on-chip cast stage at all; loads own the gpsimd ring,
                    stores own the two HWDGE rings (R/W streams overlap).
    stage 1 (PE): Y.T tiles [d, tok] via matmul(lhsT=H_tile[128tok,128d],
                  rhs=blockdiag(A,A)) - aggregation and transpose fused.
    stage 2 (DVE): evict Y.T PSUM bank to SBUF (one [128,512] copy/chunk).
    stage 3 (PE): out[tok, d_out] = Y.T.T @ Wc.T accumulated over 2 k-chunks.
    stage 4 (ACT/DVE alternating): relu + scale + PSUM->SBUF evict.
    stage 5: per-chunk DMA store, alternating HWDGE rings.

A short burst of dependency-free warm-up matmuls runs right after the
preamble barrier so the PE HAM clock gate releases (1.2 -> 2.4 GHz)
before the real matmul stream arrives.

Sharding: data-parallel over the B axis, 8 cores x 2048 rows.
"""

from contextlib import ExitStack

import numpy as np

import concourse.bacc as bacc
import concourse.bass as bass
import concourse.tile as tile
from concourse import mybir
from concourse.bass_utils import run_bass_kernel_spmd

N_AGENTS = 64
B = 256
D = 256
N_CORES = 8
ROWS = B * N_AGENTS            # 16384
ROWS_PER_CORE = ROWS // N_CORES  # 2048
P = 128
N_TILES = ROWS_PER_CORE // P   # 16 token tiles per core
# chunk plan: (tile_start, n_tiles, transport); the head and tail chunks
# ride the otherwise-idle HWDGE rings as f32 (DVE casts them) so they land
# early; the middle 12 tiles stream via SWDGE with inline f32->fp16 cast
# (bigger chunks amortize its ~1.1us per-DMA floor).
# input chunks (tile_start, n_tiles, transport): SWDGE streams the first 14
# tiles with inline f32->fp16 cast (input bandwidth is a fixed ~270GB/s pie
# - splitting transports just re-divides it); the 2 tail tiles ride the
# otherwise-idle HWDGE rings as f32 early so the drain chain is short.
CHUNKS = [(0, 2, "sw"), (2, 4, "sw"), (6, 4, "sw"), (10, 4, "sw"),
          (14, 1, "sync"), (15, 1, "scalar")]
MAXT = 4                       # max tiles per chunk (PSUM tile sizing)
# compute/store granularity: 2-tile units, decoupled from input chunking
UNITS = [(2 * u, 2) for u in range(8)]
RELU_ACT = (2, 4, 6)           # units whose relu runs on ACT (rest on DVE)
ST_SCALAR = (1, 3, 5, 7)       # units whose store issues from scalar
W_SCALE = 16.0  # fp16 weight prescale (power of 2; inverted exactly in relu)

_cache = {}


def _build(has_bias: bool, f16: bool = True):
    f32 = mybir.dt.float32
    mdt = mybir.dt.float16 if f16 else mybir.dt.float32
    inv_scale = 1.0 / W_SCALE if f16 else 1.0
    nc = bacc.Bacc("TRN2", target_bir_lowering=False, debug=False,
                   num_devices=N_CORES)

    h = nc.dram_tensor("h", [ROWS_PER_CORE, D], f32, kind="ExternalInput")
    wcT = nc.dram_tensor("wcT", [D, D], mdt, kind="ExternalInput")
    ablk = nc.dram_tensor("ablk", [P, P], mdt, kind="ExternalInput")
    if has_bias:
        bc = nc.dram_tensor("bc", [1, D], f32, kind="ExternalInput")
    out = nc.dram_tensor("out", [ROWS_PER_CORE, D], f32, kind="ExternalOutput")

    h_ap = h[:, :].rearrange("(n p) d -> p n d", p=P)      # [128, 16, 256]
    out_ap = out[:, :].rearrange("(n p) d -> p n d", p=P)  # [128, 16, 256]
    w_ap = wcT[:, :].rearrange("(k p) d -> p k d", p=P)    # [128, 2, 256]

    NCH = len(CHUNKS)

    with tile.TileContext(nc) as tc:
        with ExitStack() as ctx:
            const = ctx.enter_context(tc.tile_pool(name="const", bufs=1))
            aggps = ctx.enter_context(
                tc.tile_pool(name="aggps", bufs=2, space="PSUM"))
            outps = ctx.enter_context(
                tc.tile_pool(name="outps", bufs=3, space="PSUM"))

            a_t = const.tile([P, P], mdt, tag="a", name="a_t")
            w_t = const.tile([P, 2, D], mdt, tag="w", name="w_t")
            if has_bias:
                bc_t = const.tile([P, D], f32, tag="bc", name="bc_t")

            # ---- PE warm-up: dependency-free matmuls on scratch data so the
            # HAM clock gate releases (1.2 -> 2.4 GHz) before real work; the
            # burst bridges the input-DMA latency window (~3.4us busy needed).
            ws_t = const.tile([P, 4 * P], mdt, tag="ws", name="ws_t")
            nc.vector.memset(ws_t[:], 0.0)
            wp_t = outps.tile([P, 2, D], f32, tag="outps", name="wp_t")
            for _ in range(11):
                nc.tensor.matmul(wp_t[:], ws_t[:, :P],
                                 ws_t[:], start=True, stop=True)

            # ---- weights on the (otherwise store-only) HWDGE rings
            nc.sync.dma_start(out=a_t[:], in_=ablk[:, :])
            nc.scalar.dma_start(out=w_t[:], in_=w_ap)
            if has_bias:
                bc_bcast = bass.AP(tensor=bc, offset=0, ap=[[0, P], [1, D]])
                nc.gpsimd.dma_start(out=bc_t[:], in_=bc_bcast)

            # ---- input loads (mixed transport)
            hc = []
            traw = {}
            for c, (t0, nt, tr) in enumerate(CHUNKS):
                if tr == "sw" and f16:
                    t = const.tile([P, nt, D], mdt, tag=f"hc{c}",
                                   name=f"hc_{c}")
                    nc.gpsimd.dma_start(out=t[:], in_=h_ap[:, t0:t0 + nt, :])
                else:
                    r = const.tile([P, nt, D], f32, tag=f"hr{c}",
                                   name=f"hr_{c}")
                    eng = nc.scalar if tr == "scalar" else nc.sync
                    eng.dma_start(out=r[:], in_=h_ap[:, t0:t0 + nt, :])
                    if f16:
                        traw[c] = r
                        t = const.tile([P, nt, D], mdt, tag=f"hc{c}",
                                       name=f"hc_{c}")
                    else:
                        t = r
                hc.append(t)

            def cast(c):
                if c in traw:
                    nc.vector.tensor_copy(out=hc[c][:], in_=traw[c][:])

            # Y.T in SBUF: [128 d, 2 k-chunks, 2048 tok] single tile
            yt = const.tile([P, 2, ROWS_PER_CORE], mdt, tag="yt", name="yt")
            och = [const.tile([P, nt, D], f32, tag=f"oc{u}", name=f"oc_{u}")
                   for u, (t0, nt) in enumerate(UNITS)]

            def agg(c):
                t0, nt, _ = CHUNKS[c]
                # one PSUM region per chunk, k-major columns [k, s, 128]
                ps = aggps.tile([P, 2, MAXT * P], f32, tag="aggps",
                                name="agg_ps")
                ps = ps[:, :, :nt * P]
                for s in range(nt):
                    for k in range(2):
                        lhsT = hc[c][:, s, k * P:(k + 1) * P]
                        nc.tensor.matmul(
                            ps[:, k, s * P:(s + 1) * P], lhsT, a_t[:],
                            start=True, stop=True)
                # single DVE evict for the whole chunk (both k halves)
                nc.vector.tensor_copy(
                    out=yt[:, :, t0 * P:(t0 + nt) * P], in_=ps[:])

            def main(u):
                t0, nt = UNITS[u]
                po = outps.tile([P, 2, D], f32, tag="outps", name="po")
                po = po[:, :nt, :]
                for s in range(nt):
                    m = t0 + s
                    for k in range(2):
                        nc.tensor.matmul(
                            po[:, s, :], yt[:, k, m * P:(m + 1) * P],
                            w_t[:, k, :], start=(k == 0), stop=(k == 1))
                dst = och[u][:]
                if has_bias:
                    for s in range(nt):
                        nc.vector.tensor_scalar(
                            out=och[u][:, s, :], in0=po[:, s, :],
                            scalar1=inv_scale, scalar2=None,
                            op0=mybir.AluOpType.mult)
                        nc.vector.tensor_tensor(
                            out=och[u][:, s, :], in0=och[u][:, s, :],
                            in1=bc_t[:], op=mybir.AluOpType.add)
                        nc.scalar.activation(
                            out=och[u][:, s, :], in_=och[u][:, s, :],
                            func=mybir.ActivationFunctionType.Relu)
                elif u in RELU_ACT:
                    nc.scalar.activation(
                        out=dst, in_=po[:],
                        func=mybir.ActivationFunctionType.Relu,
                        scale=inv_scale)
                else:
                    nc.vector.tensor_scalar(
                        out=dst, in0=po[:], scalar1=inv_scale,
                        scalar2=0.0, op0=mybir.AluOpType.mult,
                        op1=mybir.AluOpType.max)
                # relus sit mostly on DVE so the two store-issue engines
                # (sync + scalar) stay responsive
                (nc.scalar if u in ST_SCALAR else nc.sync).dma_start(
                    out=out_ap[:, t0:t0 + nt, :], in_=och[u][:])

            # interleave: aggs per input chunk, mains per 2-tile unit, with
            # one-chunk lookahead so PE never waits on the DVE evicts
            agg(0)            # tiles 0-1   -> unit 0
            agg(1)            # tiles 2-5   -> units 1,2
            main(0)
            agg(2)            # tiles 6-9   -> units 3,4
            main(1)
            main(2)
            cast(4)
            cast(5)
            agg(3)            # tiles 10-13 -> units 5,6
            main(3)
            main(4)
            agg(4)            # tile 14
            agg(5)            # tile 15    -> unit 7 (with 14)
            main(5)
            main(6)
            main(7)
    nc.finalize()
    return nc


def _fold(W_act, b_act, W_sum, b_sum, W_head, b_head, f16=True):
    Wa = W_act.astype(np.float64)
    Ws = W_sum.astype(np.float64)
    Wh = W_head.astype(np.float64)
    Wc = Wh @ Ws @ Wa
    bc = (b_head.astype(np.float64)
          + b_sum.astype(np.float64) @ Wh.T
          + b_act.astype(np.float64) @ (Wh @ Ws).T)
    A = np.ones((N_AGENTS, N_AGENTS)) - np.eye(N_AGENTS)
    if f16:
        # mask stays exact 0/1 in fp16; 1/63 and the fp16-subnormal
        # prescale fold into the weights, inverted via the relu scale.
        WcT = (Wc.T / (N_AGENTS - 1) * W_SCALE).astype(np.float16)
        wdt = np.float16
    else:
        A = A / (N_AGENTS - 1)
        WcT = Wc.T.astype(np.float32)
        wdt = np.float32
    Ablk = np.zeros((P, P))
    Ablk[:N_AGENTS, :N_AGENTS] = A
    Ablk[N_AGENTS:, N_AGENTS:] = A
    return (np.ascontiguousarray(WcT), bc.astype(np.float32),
            Ablk.astype(wdt))


def kernel(hidden_state, W_act, b_act, W_sum, b_sum, W_head, b_head,
           _trace=False, _tmpdir=None):
    import os
    f16 = os.environ.get("KERNEL_F32", "0") != "1"
    h = np.ascontiguousarray(np.asarray(hidden_state, dtype=np.float32))
    WcT, bc, Ablk = _fold(np.asarray(W_act), np.asarray(b_act),
                          np.asarray(W_sum), np.asarray(b_sum),
                          np.asarray(W_head), np.asarray(b_head), f16=f16)
    has_bias = bool(np.any(bc))
    if (has_bias, f16) not in _cache:
        _cache[(has_bias, f16)] = _build(has_bias, f16=f16)
    nc = _cache[(has_bias, f16)]

    in_maps = []
    for c in range(N_CORES):
        m = {"h": h[c * ROWS_PER_CORE:(c + 1) * ROWS_PER_CORE],
             "wcT": WcT, "ablk": Ablk}
        if has_bias:
            m["bc"] = bc.reshape(1, D)
        in_maps.append(m)

    res = run_bass_kernel_spmd(
        nc, in_maps, core_ids=list(range(N_CORES)),
        trace=_trace, tmpdir=_tmpdir)
    out = np.concatenate([res.results[c]["out"] for c in range(N_CORES)],
                         axis=0)
    if _trace:
        return out, res
    return out
